# revision 20
# baseline (speedup 1.0000x reference)
"""Trainium2 Bass kernel for nn_ChannelGroupConvUneven.

Computes, for full inputs
    x      (8, 256, 128, 128) f32
    weight (320, 256, 3, 3)   f32
    bias   (320,)             f32
    param  (5,)               i32   per-group input-channel thresholds
the reference
    out = conv2d(x, weight * mask(param), stride 1, VALID) + bias
    out shape (8, 320, 126, 126) f32
where mask zeroes weight[o, i] for i < param[o // 64].

Strategy: data-parallel over batch — one image per NeuronCore (8 cores),
weights/bias replicated. Weight masking + transposition to the matmul lhsT
layout happens on the host (it is tiny, and makes the group masking exact for
any runtime `param`). Each core runs a dense 3x3 conv as PE matmuls in
bfloat16 (fp32 PSUM accumulation; measured rel err ~2e-3 vs the fp32
reference, tolerance 2e-2). bf16 weights get Fast Weight Load, so the
per-matmul LDWEIGHTS is fully hidden and matmuls pace at the 1 column/cycle
@2.4GHz roofline (~213 ns per N=504 matmul; float32r paced at ~254 ns because
its ~187 ns non-FWL LDWEIGHTS was partially exposed).

The 320 output channels are 2.5x the 128-wide PE array. Blocks A (couts
0:128) and B (128:256) run full width: per output tile (4 output rows) and
per (cin-block, dy) they take 3 matmuls each (one per kernel column dx,
rhs window x[.., dx:dx+126], PSUM-aligned to the output). The 64-wide
leftover block C (couts 256:320) would waste half the array, so its dx taps
are packed into the lhsT free dim instead:
  Cp: M=128 lanes = [64 couts @ dx=0 | 64 couts @ dx=1], rhs cols 0:127
  Cs: M=64  lanes = 64 couts @ dx=2,                     rhs cols 2:128
accumulated over (cin-block, dy) in PSUM like A/B. The dx shift is undone at
evacuation time with column-shifted adds:
  out_C[:, n] = Cp[0:64, n] + Cp[64:128, n+1] + Cs[:, n] + bias_C
as two vector-engine scalar_tensor_tensor ops. This cuts the per-(cb,dy)
PE stream from 9x504 to 6x504+508+504 cycles (-11%).

Per output tile one fp32 PSUM bank each for A, B, Cp, Cs (4 banks, so two
tiles pipeline in the 8 banks). Scalar engine evacuates A/B with a fused
per-channel bias add; vector engine does the C combine; outputs DMA out on
the sync queue. Input rows stream in bands that are double-buffered so the
PE never waits on DMA after startup.
"""

import ml_dtypes
import numpy as np

import concourse.mybir as mybir
import concourse.tile as tile
from concourse import bacc
from concourse.bass_utils import run_bass_kernel_spmd


def _ensure_axon_ntff_hook():
    """Best-effort: register the axon NTFF profile hook if the image's
    `antenv` stub lacks `axon_hooks` (concourse's trace path imports it
    unconditionally when BASS_TRACE is set). Purely optional — failures are
    ignored and tracing is simply unavailable."""
    try:
        import sys
        import types

        import antenv

        if "antenv.axon_hooks" in sys.modules:
            return
        mod = types.ModuleType("antenv.axon_hooks")
        _hook = [None]
        mod.set_axon_ntff_profile_hook = lambda h: _hook.__setitem__(0, h)
        mod.get_axon_ntff_profile_hook = lambda: _hook[0]
        sys.modules["antenv.axon_hooks"] = mod
        antenv.axon_hooks = mod
        from trn_agent_boot.trn_boot import _ntff_profile_via_ctypes

        mod.set_axon_ntff_profile_hook(
            _ntff_profile_via_ctypes("/opt/axon/libaxon_pjrt.so")
        )
    except Exception:
        pass


_ensure_axon_ntff_hook()

N_CORES = 8
P = 128
CIN, COUT, KH, KW = 256, 320, 3, 3
H = W = 128
HO = WO = 126
CB = CIN // P  # 2 cin blocks
NACC = CB * KH  # 6 PSUM-accumulated (cb, dy) passes per output tile

# output row tiles: 30 of 4 rows + 2 of 3 rows (PSUM free size <= 512 fp32
# per bank caps rpt at 4). Grouped into bands of <= 6 tiles whose input rows
# are DMA'd together (double-buffered).
TILES = [(r, 4) for r in range(0, 120, 4)] + [(120, 3), (123, 3)]
BANDS = [TILES[i : i + 6] for i in range(0, len(TILES), 6)]

MM_DT = mybir.dt.bfloat16
_NP_MM_DT = {
    mybir.dt.bfloat16: ml_dtypes.bfloat16,
    mybir.dt.float32r: np.float32,
    mybir.dt.float32: np.float32,
}

_NC_CACHE = {}


def _build_nc(mm_dt):
    nc = bacc.Bacc("TRN2", target_bir_lowering=False, debug=False)
    f32 = mybir.dt.float32
    add = mybir.AluOpType.add

    x_d = nc.dram_tensor("x", [CIN, H, W], mm_dt, kind="ExternalInput").ap()
    # A/B couts (0:256): [p, cb, dy, dx, cout]
    wab_d = nc.dram_tensor(
        "wab", [P, CB, KH, KW, 256], mm_dt, kind="ExternalInput"
    ).ap()
    # C couts packed: lanes = [64 couts @ dx0 | 64 couts @ dx1]
    wcp_d = nc.dram_tensor("wcp", [P, CB, KH, P], mm_dt, kind="ExternalInput").ap()
    # C couts @ dx2, zero-padded to 128 lanes: keeping every matmul at the
    # same 128x128 array config preserves the LDWEIGHTS pull-ahead (a 64-col
    # Cs alternating with 128-col Cp measured ~+110 ns/matmul exposed).
    wc2_d = nc.dram_tensor("wc2", [P, CB, KH, P], mm_dt, kind="ExternalInput").ap()
    b_d = nc.dram_tensor("biasp", [P, 3], f32, kind="ExternalInput").ap()
    o_d = nc.dram_tensor("out", [COUT, HO, WO], f32, kind="ExternalOutput").ap()

    # x viewed as [p, cb, h, w]: cin = cb*128 + p
    x_re = x_d.rearrange("(cb p) h w -> p cb h w", p=P)

    with tile.TileContext(nc) as tc:
        with (
            tc.tile_pool(name="wpool", bufs=1) as wpool,
            tc.tile_pool(name="xpool", bufs=3) as xpool,
            tc.tile_pool(name="opool", bufs=6) as opool,
            tc.tile_pool(name="psum", bufs=8, space="PSUM") as psum_pool,
        ):
            wab = wpool.tile([P, CB, KH, KW, 256], mm_dt)
            wcp = wpool.tile([P, CB, KH, P], mm_dt)
            wc2 = wpool.tile([P, CB, KH, P], mm_dt)
            bt = wpool.tile([P, 3], f32)

            # PE warm-spin: the HAM clock throttle needs ~3.4us of continuous
            # PE-busy inside its free-running window to step 1.2 -> 2.4 GHz.
            # The first real matmul can't issue until its weights/x rows land
            # (measured ~9.8us: engine preamble + DMA ring latency +
            # transfer), so spin the array on memset data that needs no DMA
            # starting at ~7.9us. 6 x N=512 matmuls at the cold rate end
            # ~10.5us, just after the first operands land; the busy streak
            # continues into the real matmuls and the clock steps to 2.4GHz
            # a couple of real matmuls in. (16 spins measured: PE still
            # spinning at 13.4us while data sat ready since 9.8us.)
            wsp = wpool.tile([P, 512], mm_dt)
            nc.vector.memset(wsp[:], 0.0)
            psw = psum_pool.tile([P, 512], f32, tag="ps", name="ps_spin")
            for _ in range(9):
                nc.tensor.matmul(
                    psw[:], wsp[:, 0:P], wsp[:, 0:512], start=True, stop=True
                )

            def ab_mms(ps, xb, in_r0, r, rpt, half):
                # block A (half=0) or B (half=1): 18 full-width matmuls
                co0 = half * P
                for k, (cb, dy) in enumerate(
                    (cb, dy) for cb in range(CB) for dy in range(KH)
                ):
                    rr = r - in_r0 + dy
                    for dx in range(KW):
                        nc.tensor.matmul(
                            ps[:],
                            wab[:, cb, dy, dx, co0 : co0 + P],
                            xb[:, cb, rr : rr + rpt, dx : dx + WO],
                            start=(k == 0 and dx == 0),
                            stop=(k == NACC - 1 and dx == KW - 1),
                        )

            def ab_evac(ps, r, rpt, half):
                ot = opool.tile([P, rpt, WO], f32, tag="ot")
                nc.scalar.add(ot[:], ps[:], bt[:, half : half + 1])
                nc.sync.dma_start(o_d[half * P : half * P + P, r : r + rpt, :], ot[:])

            def c_tile(xb, in_r0, r, rpt):
                # block C (couts 256:320): dx-packed matmuls + shifted combine
                pcp = psum_pool.tile([P, rpt, WO + 1], f32, tag="ps")
                pcs = psum_pool.tile([P, rpt, WO], f32, tag="ps")
                for k, (cb, dy) in enumerate(
                    (cb, dy) for cb in range(CB) for dy in range(KH)
                ):
                    rr = r - in_r0 + dy
                    nc.tensor.matmul(
                        pcp[:],
                        wcp[:, cb, dy, :],
                        xb[:, cb, rr : rr + rpt, 0 : WO + 1],
                        start=(k == 0),
                        stop=(k == NACC - 1),
                    )
                    nc.tensor.matmul(
                        pcs[:],
                        wc2[:, cb, dy, :],
                        xb[:, cb, rr : rr + rpt, 2 : 2 + WO],
                        start=(k == 0),
                        stop=(k == NACC - 1),
                    )
                # out_C = (Cp[0:64] @ dx0 + bias_C) + Cp[64:128] @ dx1 (shift 1)
                #         + Cs @ dx2, then DMA. Engines may read at most one
                #         non-scalar PSUM operand per instruction, so the dx1
                #         half goes through a scalar-engine PSUM->SBUF copy
                #         (with the partition remap 64:128 -> 0:64 and the
                #         column shift fused into the access patterns).
                tc1 = opool.tile([64, rpt, WO], f32, tag="ot")
                nc.scalar.copy(tc1[:], pcp[64:P, :, 1 : WO + 1])
                tc2 = opool.tile([64, rpt, WO], f32, tag="ot")
                nc.vector.scalar_tensor_tensor(
                    tc2[:], pcp[0:64, :, 0:WO], bt[0:64, 2:3], tc1[:],
                    op0=add, op1=add,
                )
                otc = opool.tile([64, rpt, WO], f32, tag="ot")
                nc.vector.scalar_tensor_tensor(
                    otc[:], tc2[:], 0.0, pcs[0:64], op0=add, op1=add
                )
                nc.sync.dma_start(o_d[2 * P : 2 * P + 64, r : r + rpt, :], otc[:])

            for band_idx, band in enumerate(BANDS):
                in_r0 = band[0][0]
                in_rows = band[-1][0] + band[-1][1] + 2 - in_r0
                xb = xpool.tile([P, CB, in_rows, W], mm_dt, tag="xband")
                # Band 0's input rows, the weights, and the bias are split
                # across both HWDGE queues (sync + scalar) and chunked so the
                # first tiles' matmuls start as soon as their slices land
                # (subtile deps). Queue order matters: each queue drains in
                # program order, so the first tile's needs go first. Later
                # bands prefetch on the scalar queue while output stores run
                # on sync.
                if band_idx == 0:
                    # Startup staging, ordered by first-use time. The first
                    # real matmul needs only wab[cb0,dy0,dx0] + x[cb0] rows
                    # 0:6, so those two lead the sync queue; everything the
                    # warmup's later (cb,dy) passes need streams behind on
                    # the scalar queue. Chunks stay >= 512B per partition
                    # (cout-sliced chunks degrade to 256B packets and the
                    # queue goes descriptor-bound — measured +3.7us).
                    nc.sync.dma_start(wab[:, 0, 0, 0], wab_d[:, 0, 0, 0])
                    nc.sync.dma_start(
                        xb[:, 0, 0:6], x_re[:, 0, in_r0 : in_r0 + 6, :]
                    )
                    nc.sync.dma_start(wab[:, 0, 0, 1:KW], wab_d[:, 0, 0, 1:KW])
                    nc.sync.dma_start(
                        xb[:, 0, 6:14], x_re[:, 0, in_r0 + 6 : in_r0 + 14, :]
                    )
                    nc.sync.dma_start(
                        xb[:, 0, 14:in_rows],
                        x_re[:, 0, in_r0 + 14 : in_r0 + in_rows, :],
                    )
                    nc.scalar.dma_start(bt[:], b_d[:])
                    nc.scalar.dma_start(wab[:, 0, 1], wab_d[:, 0, 1])
                    nc.scalar.dma_start(wab[:, 0, 2], wab_d[:, 0, 2])
                    nc.scalar.dma_start(wab[:, 1, 0], wab_d[:, 1, 0])
                    nc.scalar.dma_start(
                        xb[:, 1, 0:6], x_re[:, 1, in_r0 : in_r0 + 6, :]
                    )
                    nc.scalar.dma_start(
                        xb[:, 1, 6:14], x_re[:, 1, in_r0 + 6 : in_r0 + 14, :]
                    )
                    nc.scalar.dma_start(
                        xb[:, 1, 14:in_rows],
                        x_re[:, 1, in_r0 + 14 : in_r0 + in_rows, :],
                    )
                    for dy in range(1, KH):
                        nc.scalar.dma_start(wab[:, 1, dy], wab_d[:, 1, dy])
                    nc.scalar.dma_start(wcp[:], wcp_d[:])
                    nc.scalar.dma_start(wc2[:], wc2_d[:])
                else:
                    nc.scalar.dma_start(
                        xb[:], x_re[:, :, in_r0 : in_r0 + in_rows, :]
                    )

                if band_idx == 0:
                    # Warm-up sweep over block A: the weight chunks are still
                    # streaming in, and tile-major order would burn each
                    # (cb, dy) chunk in ~0.7us while chunks arrive ~2us
                    # apart. Going chunk-major across all 6 row tiles gives
                    # each chunk ~4us of work, so the PE never stalls on the
                    # weight DMA.
                    pss = [
                        psum_pool.tile([P, rpt, WO], f32, tag="ps", name=f"ps_w{ti}")
                        for ti, (r, rpt) in enumerate(band)
                    ]
                    for k, (cb, dy) in enumerate(
                        (cb, dy) for cb in range(CB) for dy in range(KH)
                    ):
                        for ti, (r, rpt) in enumerate(band):
                            rr = r - in_r0 + dy
                            for dx in range(KW):
                                nc.tensor.matmul(
                                    pss[ti][:],
                                    wab[:, cb, dy, dx, 0:P],
                                    xb[:, cb, rr : rr + rpt, dx : dx + WO],
                                    start=(k == 0 and dx == 0),
                                    stop=(k == NACC - 1 and dx == KW - 1),
                                )
                    for ti, (r, rpt) in enumerate(band):
                        ab_evac(pss[ti], r, rpt, 0)
                    for r, rpt in band:
                        ps = psum_pool.tile([P, rpt, WO], f32, tag="ps")
                        ab_mms(ps, xb, in_r0, r, rpt, 1)
                        ab_evac(ps, r, rpt, 1)
                    for r, rpt in band:
                        c_tile(xb, in_r0, r, rpt)
                elif band_idx < len(BANDS) - 1:
                    for half in range(2):
                        for r, rpt in band:
                            ps = psum_pool.tile([P, rpt, WO], f32, tag="ps")
                            ab_mms(ps, xb, in_r0, r, rpt, half)
                            ab_evac(ps, r, rpt, half)
                    for r, rpt in band:
                        c_tile(xb, in_r0, r, rpt)
                else:
                    # Last band: C tiles first so the kernel's tail after the
                    # final matmul is a single scalar bias-add + DMA (~1.1us)
                    # instead of the 4-op C combine chain (~2.3us).
                    for r, rpt in band:
                        c_tile(xb, in_r0, r, rpt)
                    for half in range(2):
                        for r, rpt in band:
                            ps = psum_pool.tile([P, rpt, WO], f32, tag="ps")
                            ab_mms(ps, xb, in_r0, r, rpt, half)
                            ab_evac(ps, r, rpt, half)
    nc.compile()
    return nc


def _get_nc():
    key = str(MM_DT)
    if key not in _NC_CACHE:
        _NC_CACHE[key] = _build_nc(MM_DT)
    return _NC_CACHE[key]


def _preprocess(x, weight, bias, param):
    np_mm = _NP_MM_DT[MM_DT]
    x = np.ascontiguousarray(np.asarray(x, dtype=np.float32).astype(np_mm))
    weight = np.asarray(weight, dtype=np.float32)
    bias = np.asarray(bias, dtype=np.float32)
    param = np.asarray(param)

    # host-side weight masking (group g of 64 output channels uses cin >= param[g])
    thresh = np.repeat(param.astype(np.int64), COUT // param.shape[0])  # [COUT]
    mask = (np.arange(CIN)[None, :] >= thresh[:, None]).astype(np.float32)
    wm = weight * mask[:, :, None, None]
    # lhsT layout: [p, cb, kh, kw, cout]
    wT = wm.reshape(COUT, CB, P, KH, KW).transpose(2, 1, 3, 4, 0).astype(np_mm)
    wab = np.ascontiguousarray(wT[..., 0:256])
    wc = wT[..., 256:320]  # [P, CB, KH, KW, 64]
    wcp = np.ascontiguousarray(
        np.concatenate([wc[:, :, :, 0, :], wc[:, :, :, 1, :]], axis=-1)
    )
    wc2 = np.zeros((P, CB, KH, P), np_mm)
    wc2[..., 0:64] = wc[:, :, :, 2, :]
    biasp = np.zeros((P, 3), np.float32)
    biasp[:, 0] = bias[0:128]
    biasp[:, 1] = bias[128:256]
    biasp[:64, 2] = bias[256:320]
    return x, wab, wcp, wc2, biasp


def kernel(x, weight, bias, param):
    x, wab, wcp, wc2, biasp = _preprocess(x, weight, bias, param)
    nc = _get_nc()
    in_maps = [
        {"x": x[i], "wab": wab, "wcp": wcp, "wc2": wc2, "biasp": biasp}
        for i in range(N_CORES)
    ]
    res = run_bass_kernel_spmd(nc, in_maps, core_ids=list(range(N_CORES)))
    return np.stack([r["out"] for r in res.results], axis=0)


# revision 24
# speedup vs baseline: 1.0083x; 1.0083x over previous
"""Trainium2 Bass kernel for nn_ChannelGroupConvUneven.

Computes, for full inputs
    x      (8, 256, 128, 128) f32
    weight (320, 256, 3, 3)   f32
    bias   (320,)             f32
    param  (5,)               i32   per-group input-channel thresholds
the reference
    out = conv2d(x, weight * mask(param), stride 1, VALID) + bias
    out shape (8, 320, 126, 126) f32
where mask zeroes weight[o, i] for i < param[o // 64].

Strategy: data-parallel over batch — one image per NeuronCore (8 cores),
weights/bias replicated. Weight masking + transposition to the matmul lhsT
layout happens on the host (it is tiny, and makes the group masking exact for
any runtime `param`). Each core runs a dense 3x3 conv as PE matmuls in
bfloat16 (fp32 PSUM accumulation; measured rel err ~2e-3 vs the fp32
reference, tolerance 2e-2). bf16 weights get Fast Weight Load, so the
per-matmul LDWEIGHTS is fully hidden and matmuls pace at the 1 column/cycle
@2.4GHz roofline (~213 ns per N=504 matmul; float32r paced at ~254 ns because
its ~187 ns non-FWL LDWEIGHTS was partially exposed).

The 320 output channels are 2.5x the 128-wide PE array. Blocks A (couts
0:128) and B (128:256) run full width: per output tile (4 output rows) and
per (cin-block, dy) they take 3 matmuls each (one per kernel column dx,
rhs window x[.., dx:dx+126], PSUM-aligned to the output). The 64-wide
leftover block C (couts 256:320) would waste half the array, so its dx taps
are packed into the lhsT free dim instead:
  Cp: M=128 lanes = [64 couts @ dx=0 | 64 couts @ dx=1], rhs cols 0:127
  Cs: M=64  lanes = 64 couts @ dx=2,                     rhs cols 2:128
accumulated over (cin-block, dy) in PSUM like A/B. The dx shift is undone at
evacuation time with column-shifted adds:
  out_C[:, n] = Cp[0:64, n] + Cp[64:128, n+1] + Cs[:, n] + bias_C
as a scalar-engine shifted copy plus two vector-engine scalar_tensor_tensor
ops. This cuts the per-(cb,dy) PE stream from 9x504 to 6x504+508+504 cycles
(-11%). Cs is zero-padded to 128 lanes so every matmul keeps the same
128x128 array config (alternating 64/128 breaks LDWEIGHTS pull-ahead).

Per output tile one fp32 PSUM bank each for A, B, Cp, Cs (4 banks, so two
tiles pipeline in the 8 banks). Scalar engine evacuates A/B with a fused
per-channel bias add; vector engine does the C combine; outputs DMA out on
the sync queue. Input rows stream in bands that are double-buffered so the
PE never waits on DMA after startup.
"""

import ml_dtypes
import numpy as np

import concourse.mybir as mybir
import concourse.tile as tile
from concourse import bacc
from concourse.bass_utils import run_bass_kernel_spmd


def _ensure_axon_ntff_hook():
    """Best-effort: register the axon NTFF profile hook if the image's
    `antenv` stub lacks `axon_hooks` (concourse's trace path imports it
    unconditionally when BASS_TRACE is set). Purely optional — failures are
    ignored and tracing is simply unavailable."""
    try:
        import sys
        import types

        import antenv

        if "antenv.axon_hooks" in sys.modules:
            return
        mod = types.ModuleType("antenv.axon_hooks")
        _hook = [None]
        mod.set_axon_ntff_profile_hook = lambda h: _hook.__setitem__(0, h)
        mod.get_axon_ntff_profile_hook = lambda: _hook[0]
        sys.modules["antenv.axon_hooks"] = mod
        antenv.axon_hooks = mod
        from trn_agent_boot.trn_boot import _ntff_profile_via_ctypes

        mod.set_axon_ntff_profile_hook(
            _ntff_profile_via_ctypes("/opt/axon/libaxon_pjrt.so")
        )
    except Exception:
        pass


_ensure_axon_ntff_hook()

N_CORES = 8
P = 128
CIN, COUT, KH, KW = 256, 320, 3, 3
H = W = 128
HO = WO = 126
CB = CIN // P  # 2 cin blocks
NACC = CB * KH  # 6 PSUM-accumulated (cb, dy) passes per output tile

# output row tiles: 30 of 4 rows + 2 of 3 rows (PSUM free size <= 512 fp32
# per bank caps rpt at 4). Grouped into bands of <= 6 tiles whose input rows
# are DMA'd together (double-buffered).
TILES = [(r, 4) for r in range(0, 120, 4)] + [(120, 3), (123, 3)]
BANDS = [TILES[i : i + 6] for i in range(0, len(TILES), 6)]

MM_DT = mybir.dt.bfloat16
_NP_MM_DT = {
    mybir.dt.bfloat16: ml_dtypes.bfloat16,
    mybir.dt.float32r: np.float32,
    mybir.dt.float32: np.float32,
}

_NC_CACHE = {}


def _build_nc(mm_dt):
    nc = bacc.Bacc("TRN2", target_bir_lowering=False, debug=False)
    f32 = mybir.dt.float32
    add = mybir.AluOpType.add

    x_d = nc.dram_tensor("x", [CIN, H, W], mm_dt, kind="ExternalInput").ap()
    # A/B couts (0:256): [p, cb, dy, dx, cout]
    wab_d = nc.dram_tensor(
        "wab", [P, CB, KH, KW, 256], mm_dt, kind="ExternalInput"
    ).ap()
    # C couts packed: lanes = [64 couts @ dx0 | 64 couts @ dx1]
    wcp_d = nc.dram_tensor("wcp", [P, CB, KH, P], mm_dt, kind="ExternalInput").ap()
    # C couts @ dx2, zero-padded to 128 lanes: keeping every matmul at the
    # same 128x128 array config preserves the LDWEIGHTS pull-ahead (a 64-col
    # Cs alternating with 128-col Cp measured ~+110 ns/matmul exposed).
    wc2_d = nc.dram_tensor("wc2", [P, CB, KH, P], mm_dt, kind="ExternalInput").ap()
    b_d = nc.dram_tensor("biasp", [P, 3], f32, kind="ExternalInput").ap()
    o_d = nc.dram_tensor("out", [COUT, HO, WO], f32, kind="ExternalOutput").ap()

    # x viewed as [p, cb, h, w]: cin = cb*128 + p
    x_re = x_d.rearrange("(cb p) h w -> p cb h w", p=P)

    with tile.TileContext(nc) as tc:
        with (
            tc.tile_pool(name="wpool", bufs=1) as wpool,
            tc.tile_pool(name="xpool", bufs=3) as xpool,
            tc.tile_pool(name="opool", bufs=6) as opool,
            tc.tile_pool(name="psum", bufs=8, space="PSUM") as psum_pool,
        ):
            wab = wpool.tile([P, CB, KH, KW, 256], mm_dt)
            wcp = wpool.tile([P, CB, KH, P], mm_dt)
            wc2 = wpool.tile([P, CB, KH, P], mm_dt)
            bt = wpool.tile([P, 3], f32)

            # PE warm-spin: the HAM clock throttle needs ~3.4us of continuous
            # PE-busy inside its free-running window to step 1.2 -> 2.4 GHz.
            # The first real matmul can't issue until the warmup's weights
            # and x band land (~13us: engine preamble + DMA ring latency +
            # transfer + completion-signal lag), so spin the array on memset
            # data that needs no DMA, starting at ~7.9us. 16 N=512 matmuls
            # (cold rate first ~8, then warm) bridge the PE to ~13.4us at
            # full clock, so the real stream starts warm and gapless.
            # Measured alternatives: fewer spins end before the data and the
            # idle gap re-throttles the clock (K=4/8) for the first ~12 real
            # matmuls; starting real matmuls earlier on finer x chunks stalls
            # mid-warmup instead, with the same re-throttle.
            wsp = wpool.tile([P, 512], mm_dt)
            nc.vector.memset(wsp[:], 0.0)
            psw = psum_pool.tile([P, 512], f32, tag="ps", name="ps_spin")
            for _ in range(16):
                nc.tensor.matmul(
                    psw[:], wsp[:, 0:P], wsp[:, 0:512], start=True, stop=True
                )

            def ab_mms(ps, xb, in_r0, r, rpt, half):
                # block A (half=0) or B (half=1): 18 full-width matmuls
                co0 = half * P
                for k, (cb, dy) in enumerate(
                    (cb, dy) for cb in range(CB) for dy in range(KH)
                ):
                    rr = r - in_r0 + dy
                    for dx in range(KW):
                        nc.tensor.matmul(
                            ps[:],
                            wab[:, cb, dy, dx, co0 : co0 + P],
                            xb[:, cb, rr : rr + rpt, dx : dx + WO],
                            start=(k == 0 and dx == 0),
                            stop=(k == NACC - 1 and dx == KW - 1),
                        )

            def ab_evac(ps, r, rpt, half):
                ot = opool.tile([P, rpt, WO], f32, tag="ot")
                nc.scalar.add(ot[:], ps[:], bt[:, half : half + 1])
                nc.sync.dma_start(o_d[half * P : half * P + P, r : r + rpt, :], ot[:])

            def c_tile(xb, in_r0, r, rpt):
                # block C (couts 256:320): dx-packed matmuls + shifted combine
                pcp = psum_pool.tile([P, rpt, WO + 1], f32, tag="ps")
                pcs = psum_pool.tile([P, rpt, WO], f32, tag="ps")
                for k, (cb, dy) in enumerate(
                    (cb, dy) for cb in range(CB) for dy in range(KH)
                ):
                    rr = r - in_r0 + dy
                    nc.tensor.matmul(
                        pcp[:],
                        wcp[:, cb, dy, :],
                        xb[:, cb, rr : rr + rpt, 0 : WO + 1],
                        start=(k == 0),
                        stop=(k == NACC - 1),
                    )
                    nc.tensor.matmul(
                        pcs[:],
                        wc2[:, cb, dy, :],
                        xb[:, cb, rr : rr + rpt, 2 : 2 + WO],
                        start=(k == 0),
                        stop=(k == NACC - 1),
                    )
                # out_C = (Cp[0:64] @ dx0 + bias_C) + Cp[64:128] @ dx1 (shift 1)
                #         + Cs @ dx2, then DMA. Engines may read at most one
                #         non-scalar PSUM operand per instruction, so the dx1
                #         half goes through a scalar-engine PSUM->SBUF copy
                #         (with the partition remap 64:128 -> 0:64 and the
                #         column shift fused into the access patterns).
                tc1 = opool.tile([64, rpt, WO], f32, tag="ot")
                nc.scalar.copy(tc1[:], pcp[64:P, :, 1 : WO + 1])
                tc2 = opool.tile([64, rpt, WO], f32, tag="ot")
                nc.vector.scalar_tensor_tensor(
                    tc2[:], pcp[0:64, :, 0:WO], bt[0:64, 2:3], tc1[:],
                    op0=add, op1=add,
                )
                otc = opool.tile([64, rpt, WO], f32, tag="ot")
                nc.vector.scalar_tensor_tensor(
                    otc[:], tc2[:], 0.0, pcs[0:64], op0=add, op1=add
                )
                nc.sync.dma_start(o_d[2 * P : 2 * P + 64, r : r + rpt, :], otc[:])

            for band_idx, band in enumerate(BANDS):
                in_r0 = band[0][0]
                in_rows = band[-1][0] + band[-1][1] + 2 - in_r0
                xb = xpool.tile([P, CB, in_rows, W], mm_dt, tag="xband")
                # Later bands prefetch on the scalar queue while output
                # stores run on sync. Each queue drains in program order, so
                # band 0's transfers are ordered by first use (subtile deps
                # let matmuls start as soon as their slices land).
                if band_idx == 0:
                    # Startup staging across both HWDGE queues, chunked so
                    # the warmup's needs land in first-use order. Weight
                    # chunks keep the full 256-cout width: per-partition that
                    # is a contiguous 1536B DMA packet, where a cout-sliced
                    # chunk degrades to 256B packets and the whole queue goes
                    # descriptor-bound (measured +3.7us on everything queued
                    # behind it).
                    for cb in range(CB):
                        eng = nc.sync if cb == 0 else nc.scalar
                        eng.dma_start(
                            xb[:, cb, 0:6], x_re[:, cb, in_r0 : in_r0 + 6, :]
                        )
                    nc.scalar.dma_start(bt[:], b_d[:])
                    nc.sync.dma_start(wab[:, 0, 0], wab_d[:, 0, 0])
                    nc.scalar.dma_start(wab[:, 1, 0], wab_d[:, 1, 0])
                    for cb in range(CB):
                        eng = nc.sync if cb == 0 else nc.scalar
                        eng.dma_start(
                            xb[:, cb, 6:14], x_re[:, cb, in_r0 + 6 : in_r0 + 14, :]
                        )
                        eng.dma_start(
                            xb[:, cb, 14:in_rows],
                            x_re[:, cb, in_r0 + 14 : in_r0 + in_rows, :],
                        )
                    for dy in range(1, KH):
                        nc.sync.dma_start(wab[:, 0, dy], wab_d[:, 0, dy])
                        nc.scalar.dma_start(wab[:, 1, dy], wab_d[:, 1, dy])
                    nc.scalar.dma_start(wcp[:], wcp_d[:])
                    nc.scalar.dma_start(wc2[:], wc2_d[:])
                else:
                    nc.scalar.dma_start(
                        xb[:], x_re[:, :, in_r0 : in_r0 + in_rows, :]
                    )

                if band_idx == 0:
                    # Warm-up sweep over block A: the weight chunks are still
                    # streaming in, and tile-major order would burn each
                    # (cb, dy) chunk in ~0.7us while chunks arrive ~2us
                    # apart. Going chunk-major across all 6 row tiles gives
                    # each chunk ~4us of work, so the PE never stalls on the
                    # weight DMA.
                    pss = [
                        psum_pool.tile([P, rpt, WO], f32, tag="ps", name=f"ps_w{ti}")
                        for ti, (r, rpt) in enumerate(band)
                    ]
                    for k, (cb, dy) in enumerate(
                        (cb, dy) for cb in range(CB) for dy in range(KH)
                    ):
                        for ti, (r, rpt) in enumerate(band):
                            rr = r - in_r0 + dy
                            for dx in range(KW):
                                nc.tensor.matmul(
                                    pss[ti][:],
                                    wab[:, cb, dy, dx, 0:P],
                                    xb[:, cb, rr : rr + rpt, dx : dx + WO],
                                    start=(k == 0 and dx == 0),
                                    stop=(k == NACC - 1 and dx == KW - 1),
                                )
                    for ti, (r, rpt) in enumerate(band):
                        ab_evac(pss[ti], r, rpt, 0)
                    for r, rpt in band:
                        ps = psum_pool.tile([P, rpt, WO], f32, tag="ps")
                        ab_mms(ps, xb, in_r0, r, rpt, 1)
                        ab_evac(ps, r, rpt, 1)
                    for r, rpt in band:
                        c_tile(xb, in_r0, r, rpt)
                elif band_idx < len(BANDS) - 1:
                    for half in range(2):
                        for r, rpt in band:
                            ps = psum_pool.tile([P, rpt, WO], f32, tag="ps")
                            ab_mms(ps, xb, in_r0, r, rpt, half)
                            ab_evac(ps, r, rpt, half)
                    for r, rpt in band:
                        c_tile(xb, in_r0, r, rpt)
                else:
                    # Last band: C tiles first so the kernel's tail after the
                    # final matmul is a single scalar bias-add + DMA (~1.1us)
                    # instead of the 4-op C combine chain (~2.3us).
                    for r, rpt in band:
                        c_tile(xb, in_r0, r, rpt)
                    for half in range(2):
                        for r, rpt in band:
                            ps = psum_pool.tile([P, rpt, WO], f32, tag="ps")
                            ab_mms(ps, xb, in_r0, r, rpt, half)
                            ab_evac(ps, r, rpt, half)
    nc.compile()
    return nc


def _get_nc():
    key = str(MM_DT)
    if key not in _NC_CACHE:
        _NC_CACHE[key] = _build_nc(MM_DT)
    return _NC_CACHE[key]


def _preprocess(x, weight, bias, param):
    np_mm = _NP_MM_DT[MM_DT]
    x = np.ascontiguousarray(np.asarray(x, dtype=np.float32).astype(np_mm))
    weight = np.asarray(weight, dtype=np.float32)
    bias = np.asarray(bias, dtype=np.float32)
    param = np.asarray(param)

    # host-side weight masking (group g of 64 output channels uses cin >= param[g])
    thresh = np.repeat(param.astype(np.int64), COUT // param.shape[0])  # [COUT]
    mask = (np.arange(CIN)[None, :] >= thresh[:, None]).astype(np.float32)
    wm = weight * mask[:, :, None, None]
    # lhsT layout: [p, cb, kh, kw, cout]
    wT = wm.reshape(COUT, CB, P, KH, KW).transpose(2, 1, 3, 4, 0).astype(np_mm)
    wab = np.ascontiguousarray(wT[..., 0:256])
    wc = wT[..., 256:320]  # [P, CB, KH, KW, 64]
    wcp = np.ascontiguousarray(
        np.concatenate([wc[:, :, :, 0, :], wc[:, :, :, 1, :]], axis=-1)
    )
    wc2 = np.zeros((P, CB, KH, P), np_mm)
    wc2[..., 0:64] = wc[:, :, :, 2, :]
    biasp = np.zeros((P, 3), np.float32)
    biasp[:, 0] = bias[0:128]
    biasp[:, 1] = bias[128:256]
    biasp[:64, 2] = bias[256:320]
    return x, wab, wcp, wc2, biasp


def kernel(x, weight, bias, param):
    x, wab, wcp, wc2, biasp = _preprocess(x, weight, bias, param)
    nc = _get_nc()
    in_maps = [
        {"x": x[i], "wab": wab, "wcp": wcp, "wc2": wc2, "biasp": biasp}
        for i in range(N_CORES)
    ]
    res = run_bass_kernel_spmd(nc, in_maps, core_ids=list(range(N_CORES)))
    return np.stack([r["out"] for r in res.results], axis=0)


# revision 26
# speedup vs baseline: 1.0104x; 1.0020x over previous
"""Trainium2 Bass kernel for nn_ChannelGroupConvUneven.

Computes, for full inputs
    x      (8, 256, 128, 128) f32
    weight (320, 256, 3, 3)   f32
    bias   (320,)             f32
    param  (5,)               i32   per-group input-channel thresholds
the reference
    out = conv2d(x, weight * mask(param), stride 1, VALID) + bias
    out shape (8, 320, 126, 126) f32
where mask zeroes weight[o, i] for i < param[o // 64].

Strategy: data-parallel over batch — one image per NeuronCore (8 cores),
weights/bias replicated. Weight masking + transposition to the matmul lhsT
layout happens on the host (it is tiny, and makes the group masking exact for
any runtime `param`). Each core runs a dense 3x3 conv as PE matmuls in
bfloat16 (fp32 PSUM accumulation; measured rel err ~2e-3 vs the fp32
reference, tolerance 2e-2). bf16 weights get Fast Weight Load, so the
per-matmul LDWEIGHTS is fully hidden and matmuls pace at the 1 column/cycle
@2.4GHz roofline (~213 ns per N=504 matmul; float32r paced at ~254 ns because
its ~187 ns non-FWL LDWEIGHTS was partially exposed).

The 320 output channels are 2.5x the 128-wide PE array. Blocks A (couts
0:128) and B (128:256) run full width: per output tile (4 output rows) and
per (cin-block, dy) they take 3 matmuls each (one per kernel column dx,
rhs window x[.., dx:dx+126], PSUM-aligned to the output). The 64-wide
leftover block C (couts 256:320) would waste half the array, so its dx taps
are packed into the lhsT free dim instead:
  Cp: M=128 lanes = [64 couts @ dx=0 | 64 couts @ dx=1], rhs cols 0:127
  Cs: M=64  lanes = 64 couts @ dx=2,                     rhs cols 2:128
accumulated over (cin-block, dy) in PSUM like A/B. The dx shift is undone at
evacuation time with column-shifted adds:
  out_C[:, n] = Cp[0:64, n] + Cp[64:128, n+1] + Cs[:, n] + bias_C
as a scalar-engine shifted copy plus two vector-engine scalar_tensor_tensor
ops. This cuts the per-(cb,dy) PE stream from 9x504 to 6x504+508+504 cycles
(-11%). Cs is zero-padded to 128 lanes so every matmul keeps the same
128x128 array config (alternating 64/128 breaks LDWEIGHTS pull-ahead).

Per output tile one fp32 PSUM bank each for A, B, Cp, Cs (4 banks, so two
tiles pipeline in the 8 banks). Scalar engine evacuates A/B with a fused
per-channel bias add; vector engine does the C combine; outputs DMA out on
the sync queue. Input rows stream in bands that are double-buffered so the
PE never waits on DMA after startup.
"""

import ml_dtypes
import numpy as np

import concourse.mybir as mybir
import concourse.tile as tile
from concourse import bacc
from concourse.bass_utils import run_bass_kernel_spmd


def _ensure_axon_ntff_hook():
    """Best-effort: register the axon NTFF profile hook if the image's
    `antenv` stub lacks `axon_hooks` (concourse's trace path imports it
    unconditionally when BASS_TRACE is set). Purely optional — failures are
    ignored and tracing is simply unavailable."""
    try:
        import sys
        import types

        import antenv

        if "antenv.axon_hooks" in sys.modules:
            return
        mod = types.ModuleType("antenv.axon_hooks")
        _hook = [None]
        mod.set_axon_ntff_profile_hook = lambda h: _hook.__setitem__(0, h)
        mod.get_axon_ntff_profile_hook = lambda: _hook[0]
        sys.modules["antenv.axon_hooks"] = mod
        antenv.axon_hooks = mod
        from trn_agent_boot.trn_boot import _ntff_profile_via_ctypes

        mod.set_axon_ntff_profile_hook(
            _ntff_profile_via_ctypes("/opt/axon/libaxon_pjrt.so")
        )
    except Exception:
        pass


_ensure_axon_ntff_hook()

N_CORES = 8
P = 128
CIN, COUT, KH, KW = 256, 320, 3, 3
H = W = 128
HO = WO = 126
CB = CIN // P  # 2 cin blocks
NACC = CB * KH  # 6 PSUM-accumulated (cb, dy) passes per output tile

# output row tiles: 30 of 4 rows + 2 of 3 rows (PSUM free size <= 512 fp32
# per bank caps rpt at 4). Grouped into bands of <= 6 tiles whose input rows
# are DMA'd together (double-buffered).
TILES = [(r, 4) for r in range(0, 120, 4)] + [(120, 3), (123, 3)]
BANDS = [TILES[i : i + 6] for i in range(0, len(TILES), 6)]

MM_DT = mybir.dt.bfloat16
_NP_MM_DT = {
    mybir.dt.bfloat16: ml_dtypes.bfloat16,
    mybir.dt.float32r: np.float32,
    mybir.dt.float32: np.float32,
}

_NC_CACHE = {}


def _build_nc(mm_dt):
    nc = bacc.Bacc("TRN2", target_bir_lowering=False, debug=False)
    f32 = mybir.dt.float32
    add = mybir.AluOpType.add

    x_d = nc.dram_tensor("x", [CIN, H, W], mm_dt, kind="ExternalInput").ap()
    # A/B couts (0:256): [p, cb, dy, dx, cout]
    wab_d = nc.dram_tensor(
        "wab", [P, CB, KH, KW, 256], mm_dt, kind="ExternalInput"
    ).ap()
    # C couts packed: lanes = [64 couts @ dx0 | 64 couts @ dx1]
    wcp_d = nc.dram_tensor("wcp", [P, CB, KH, P], mm_dt, kind="ExternalInput").ap()
    # C couts @ dx2, zero-padded to 128 lanes: keeping every matmul at the
    # same 128x128 array config preserves the LDWEIGHTS pull-ahead (a 64-col
    # Cs alternating with 128-col Cp measured ~+110 ns/matmul exposed).
    wc2_d = nc.dram_tensor("wc2", [P, CB, KH, P], mm_dt, kind="ExternalInput").ap()
    b_d = nc.dram_tensor("biasp", [P, 3], f32, kind="ExternalInput").ap()
    o_d = nc.dram_tensor("out", [COUT, HO, WO], f32, kind="ExternalOutput").ap()

    # x viewed as [p, cb, h, w]: cin = cb*128 + p
    x_re = x_d.rearrange("(cb p) h w -> p cb h w", p=P)

    with tile.TileContext(nc) as tc:
        with (
            tc.tile_pool(name="wpool", bufs=1) as wpool,
            tc.tile_pool(name="xpool", bufs=3) as xpool,
            tc.tile_pool(name="opool", bufs=6) as opool,
            tc.tile_pool(name="psum", bufs=8, space="PSUM") as psum_pool,
        ):
            wab = wpool.tile([P, CB, KH, KW, 256], mm_dt)
            wcp = wpool.tile([P, CB, KH, P], mm_dt)
            wc2 = wpool.tile([P, CB, KH, P], mm_dt)
            bt = wpool.tile([P, 3], f32)

            # PE warm-spin: the HAM clock throttle needs ~3.4us of continuous
            # PE-busy inside its free-running window to step 1.2 -> 2.4 GHz.
            # The first real matmul can't issue until the warmup's weights
            # and x band land (~13us: engine preamble + DMA ring latency +
            # transfer + completion-signal lag), so spin the array on memset
            # data that needs no DMA, starting at ~7.9us. 16 N=512 matmuls
            # (cold rate first ~8, then warm) bridge the PE to ~13.4us at
            # full clock, so the real stream starts warm and gapless.
            # Measured alternatives: fewer spins end before the data and the
            # idle gap re-throttles the clock (K=4/8) for the first ~12 real
            # matmuls; starting real matmuls earlier on finer x chunks stalls
            # mid-warmup instead, with the same re-throttle.
            wsp = wpool.tile([P, 512], mm_dt)
            nc.vector.memset(wsp[:], 0.0)
            psw = psum_pool.tile([P, 512], f32, tag="ps", name="ps_spin")
            for _ in range(16):
                nc.tensor.matmul(
                    psw[:], wsp[:, 0:P], wsp[:, 0:512], start=True, stop=True
                )

            def ab_mms(ps, xb, in_r0, r, rpt, half):
                # block A (half=0) or B (half=1): 18 full-width matmuls
                co0 = half * P
                for k, (cb, dy) in enumerate(
                    (cb, dy) for cb in range(CB) for dy in range(KH)
                ):
                    rr = r - in_r0 + dy
                    for dx in range(KW):
                        nc.tensor.matmul(
                            ps[:],
                            wab[:, cb, dy, dx, co0 : co0 + P],
                            xb[:, cb, rr : rr + rpt, dx : dx + WO],
                            start=(k == 0 and dx == 0),
                            stop=(k == NACC - 1 and dx == KW - 1),
                        )

            def ab_evac(ps, r, rpt, half, split=False):
                ot = opool.tile([P, rpt, WO], f32, tag="ot")
                nc.scalar.add(ot[:], ps[:], bt[:, half : half + 1])
                co0 = half * P
                if split:
                    # Final tile only: its output store is the kernel's tail
                    # (nothing left to overlap with), so halve the transfer
                    # by splitting across both HWDGE queues (~0.8us).
                    nc.sync.dma_start(o_d[co0 : co0 + 64, r : r + rpt, :], ot[0:64])
                    nc.scalar.dma_start(
                        o_d[co0 + 64 : co0 + P, r : r + rpt, :], ot[64:P]
                    )
                else:
                    nc.sync.dma_start(o_d[co0 : co0 + P, r : r + rpt, :], ot[:])

            def c_tile(xb, in_r0, r, rpt):
                # block C (couts 256:320): dx-packed matmuls + shifted combine
                pcp = psum_pool.tile([P, rpt, WO + 1], f32, tag="ps")
                pcs = psum_pool.tile([P, rpt, WO], f32, tag="ps")
                for k, (cb, dy) in enumerate(
                    (cb, dy) for cb in range(CB) for dy in range(KH)
                ):
                    rr = r - in_r0 + dy
                    nc.tensor.matmul(
                        pcp[:],
                        wcp[:, cb, dy, :],
                        xb[:, cb, rr : rr + rpt, 0 : WO + 1],
                        start=(k == 0),
                        stop=(k == NACC - 1),
                    )
                    nc.tensor.matmul(
                        pcs[:],
                        wc2[:, cb, dy, :],
                        xb[:, cb, rr : rr + rpt, 2 : 2 + WO],
                        start=(k == 0),
                        stop=(k == NACC - 1),
                    )
                # out_C = (Cp[0:64] @ dx0 + bias_C) + Cp[64:128] @ dx1 (shift 1)
                #         + Cs @ dx2, then DMA. Engines may read at most one
                #         non-scalar PSUM operand per instruction, so the dx1
                #         half goes through a scalar-engine PSUM->SBUF copy
                #         (with the partition remap 64:128 -> 0:64 and the
                #         column shift fused into the access patterns).
                tc1 = opool.tile([64, rpt, WO], f32, tag="ot")
                nc.scalar.copy(tc1[:], pcp[64:P, :, 1 : WO + 1])
                tc2 = opool.tile([64, rpt, WO], f32, tag="ot")
                nc.vector.scalar_tensor_tensor(
                    tc2[:], pcp[0:64, :, 0:WO], bt[0:64, 2:3], tc1[:],
                    op0=add, op1=add,
                )
                otc = opool.tile([64, rpt, WO], f32, tag="ot")
                nc.vector.scalar_tensor_tensor(
                    otc[:], tc2[:], 0.0, pcs[0:64], op0=add, op1=add
                )
                nc.sync.dma_start(o_d[2 * P : 2 * P + 64, r : r + rpt, :], otc[:])

            for band_idx, band in enumerate(BANDS):
                in_r0 = band[0][0]
                in_rows = band[-1][0] + band[-1][1] + 2 - in_r0
                xb = xpool.tile([P, CB, in_rows, W], mm_dt, tag="xband")
                # Later bands prefetch on the scalar queue while output
                # stores run on sync. Each queue drains in program order, so
                # band 0's transfers are ordered by first use (subtile deps
                # let matmuls start as soon as their slices land).
                if band_idx == 0:
                    # Startup staging across both HWDGE queues, chunked so
                    # the warmup's needs land in first-use order. Weight
                    # chunks keep the full 256-cout width: per-partition that
                    # is a contiguous 1536B DMA packet, where a cout-sliced
                    # chunk degrades to 256B packets and the whole queue goes
                    # descriptor-bound (measured +3.7us on everything queued
                    # behind it).
                    for cb in range(CB):
                        eng = nc.sync if cb == 0 else nc.scalar
                        eng.dma_start(
                            xb[:, cb, 0:6], x_re[:, cb, in_r0 : in_r0 + 6, :]
                        )
                    nc.scalar.dma_start(bt[:], b_d[:])
                    nc.sync.dma_start(wab[:, 0, 0], wab_d[:, 0, 0])
                    nc.scalar.dma_start(wab[:, 1, 0], wab_d[:, 1, 0])
                    for cb in range(CB):
                        eng = nc.sync if cb == 0 else nc.scalar
                        eng.dma_start(
                            xb[:, cb, 6:14], x_re[:, cb, in_r0 + 6 : in_r0 + 14, :]
                        )
                        eng.dma_start(
                            xb[:, cb, 14:in_rows],
                            x_re[:, cb, in_r0 + 14 : in_r0 + in_rows, :],
                        )
                    for dy in range(1, KH):
                        nc.sync.dma_start(wab[:, 0, dy], wab_d[:, 0, dy])
                        nc.scalar.dma_start(wab[:, 1, dy], wab_d[:, 1, dy])
                    nc.scalar.dma_start(wcp[:], wcp_d[:])
                    nc.scalar.dma_start(wc2[:], wc2_d[:])
                else:
                    nc.scalar.dma_start(
                        xb[:], x_re[:, :, in_r0 : in_r0 + in_rows, :]
                    )

                if band_idx == 0:
                    # Warm-up sweep over block A: the weight chunks are still
                    # streaming in, and tile-major order would burn each
                    # (cb, dy) chunk in ~0.7us while chunks arrive ~2us
                    # apart. Going chunk-major across all 6 row tiles gives
                    # each chunk ~4us of work, so the PE never stalls on the
                    # weight DMA.
                    pss = [
                        psum_pool.tile([P, rpt, WO], f32, tag="ps", name=f"ps_w{ti}")
                        for ti, (r, rpt) in enumerate(band)
                    ]
                    for k, (cb, dy) in enumerate(
                        (cb, dy) for cb in range(CB) for dy in range(KH)
                    ):
                        for ti, (r, rpt) in enumerate(band):
                            rr = r - in_r0 + dy
                            for dx in range(KW):
                                nc.tensor.matmul(
                                    pss[ti][:],
                                    wab[:, cb, dy, dx, 0:P],
                                    xb[:, cb, rr : rr + rpt, dx : dx + WO],
                                    start=(k == 0 and dx == 0),
                                    stop=(k == NACC - 1 and dx == KW - 1),
                                )
                    for ti, (r, rpt) in enumerate(band):
                        ab_evac(pss[ti], r, rpt, 0)
                    for r, rpt in band:
                        ps = psum_pool.tile([P, rpt, WO], f32, tag="ps")
                        ab_mms(ps, xb, in_r0, r, rpt, 1)
                        ab_evac(ps, r, rpt, 1)
                    for r, rpt in band:
                        c_tile(xb, in_r0, r, rpt)
                elif band_idx < len(BANDS) - 1:
                    for half in range(2):
                        for r, rpt in band:
                            ps = psum_pool.tile([P, rpt, WO], f32, tag="ps")
                            ab_mms(ps, xb, in_r0, r, rpt, half)
                            ab_evac(ps, r, rpt, half)
                    for r, rpt in band:
                        c_tile(xb, in_r0, r, rpt)
                else:
                    # Last band: C tiles first so the kernel's tail after the
                    # final matmul is a single scalar bias-add + DMA (~1.1us)
                    # instead of the 4-op C combine chain (~2.3us).
                    for r, rpt in band:
                        c_tile(xb, in_r0, r, rpt)
                    for half in range(2):
                        for tix, (r, rpt) in enumerate(band):
                            last = half == 1 and tix == len(band) - 1
                            ps = psum_pool.tile([P, rpt, WO], f32, tag="ps")
                            ab_mms(ps, xb, in_r0, r, rpt, half)
                            ab_evac(ps, r, rpt, half, split=last)
    nc.compile()
    return nc


def _get_nc():
    key = str(MM_DT)
    if key not in _NC_CACHE:
        _NC_CACHE[key] = _build_nc(MM_DT)
    return _NC_CACHE[key]


def _preprocess(x, weight, bias, param):
    np_mm = _NP_MM_DT[MM_DT]
    x = np.ascontiguousarray(np.asarray(x, dtype=np.float32).astype(np_mm))
    weight = np.asarray(weight, dtype=np.float32)
    bias = np.asarray(bias, dtype=np.float32)
    param = np.asarray(param)

    # host-side weight masking (group g of 64 output channels uses cin >= param[g])
    thresh = np.repeat(param.astype(np.int64), COUT // param.shape[0])  # [COUT]
    mask = (np.arange(CIN)[None, :] >= thresh[:, None]).astype(np.float32)
    wm = weight * mask[:, :, None, None]
    # lhsT layout: [p, cb, kh, kw, cout]
    wT = wm.reshape(COUT, CB, P, KH, KW).transpose(2, 1, 3, 4, 0).astype(np_mm)
    wab = np.ascontiguousarray(wT[..., 0:256])
    wc = wT[..., 256:320]  # [P, CB, KH, KW, 64]
    wcp = np.ascontiguousarray(
        np.concatenate([wc[:, :, :, 0, :], wc[:, :, :, 1, :]], axis=-1)
    )
    wc2 = np.zeros((P, CB, KH, P), np_mm)
    wc2[..., 0:64] = wc[:, :, :, 2, :]
    biasp = np.zeros((P, 3), np.float32)
    biasp[:, 0] = bias[0:128]
    biasp[:, 1] = bias[128:256]
    biasp[:64, 2] = bias[256:320]
    return x, wab, wcp, wc2, biasp


def kernel(x, weight, bias, param):
    x, wab, wcp, wc2, biasp = _preprocess(x, weight, bias, param)
    nc = _get_nc()
    in_maps = [
        {"x": x[i], "wab": wab, "wcp": wcp, "wc2": wc2, "biasp": biasp}
        for i in range(N_CORES)
    ]
    res = run_bass_kernel_spmd(nc, in_maps, core_ids=list(range(N_CORES)))
    return np.stack([r["out"] for r in res.results], axis=0)
